# revision 6
# baseline (speedup 1.0000x reference)
"""MoE (DeepSeek-style) routed+shared expert forward on 8 TRN2 NeuronCores.

Strategy (expert-parallel, host-side dispatch):
  - Host computes the gate (softmax + top-2) in float64 and gathers each
    expert's routed tokens (this is the "all-to-all by routing index" --
    with full inputs on the host, the host does the dispatch).
  - Core e processes expert e's routed tokens (padded to a uniform
    capacity C) through the SwiGLU FFN, plus a 1/8 slice of all tokens
    through the replicated shared-expert MLP.
  - All activations/weights are fed transposed (features on SBUF
    partitions, tokens on the free dim) so the w1/w3 -> swiglu -> w2
    chain needs no on-chip transposes.
  - Matmuls use float32r (full-rate fp32 on the PE array).
  - Host scatters expert outputs back by routing index, scales by the
    gate weights, and adds the shared-expert output.
"""

import sys

if "/opt/trn_rl_repo" not in sys.path:
    sys.path.insert(0, "/opt/trn_rl_repo")

import numpy as np

import concourse.bass as bass
import concourse.tile as tile
from concourse import bacc, mybir
from concourse import bass_utils

B, S, DIM = 4, 2048, 1024
T = B * S
INTER = 1024
E = 8
TOPK = 2
ROUTE_SCALE = 1.0
SHARED_INTER = 2048
N_CORES = 8
TOKS_SHARED = T // N_CORES  # shared-expert tokens per core
BLK = 512

F32 = mybir.dt.float32
F32R = mybir.dt.float32r
SILU = mybir.ActivationFunctionType.Silu
IDENT = mybir.ActivationFunctionType.Identity

_program_cache = {}


def _blocks(total):
    out = []
    o = 0
    while o < total:
        n = min(BLK, total - o)
        out.append((o, n))
        o += n
    return out


def build_program(C):
    """Build the per-core SPMD Bass program for routed capacity C."""
    nc = bacc.Bacc("TRN2", target_bir_lowering=False, debug=False,
                   num_devices=N_CORES)

    def din(name, shape, dt=F32):
        return nc.dram_tensor(name, shape, dt, kind="ExternalInput").ap()

    def dout(name, shape):
        return nc.dram_tensor(name, shape, F32, kind="ExternalOutput").ap()

    xe = din("xe", (DIM, C), F32R)            # routed tokens for this core's expert, transposed
    xs = din("xs", (DIM, TOKS_SHARED), F32R)  # this core's shared-token slice, transposed
    w1t = din("w1t", (DIM, INTER), F32R)      # w1[e].T
    w3t = din("w3t", (DIM, INTER), F32R)
    w2t = din("w2t", (INTER, DIM), F32R)      # w2[e].T
    ws1t = din("ws1t", (DIM, SHARED_INTER), F32R)
    ws3t = din("ws3t", (DIM, SHARED_INTER), F32R)
    ws2t = din("ws2t", (SHARED_INTER, DIM), F32R)
    b1 = din("b1", (INTER,))
    b3 = din("b3", (INTER,))
    b2 = din("b2", (DIM,))
    bs1 = din("bs1", (SHARED_INTER,))
    bs3 = din("bs3", (SHARED_INTER,))
    bs2 = din("bs2", (DIM,))
    ye = dout("ye", (DIM, C))
    ys = dout("ys", (DIM, TOKS_SHARED))

    ND = DIM // 128        # 8 k-tiles over DIM
    NI = INTER // 128      # 8 tiles over INTER
    NS = SHARED_INTER // 128  # 16 tiles over SHARED_INTER

    xe_r = xe.rearrange("(dk p) c -> p dk c", p=128)
    xs_r = xs.rearrange("(dk p) c -> p dk c", p=128)
    w1_r = w1t.rearrange("(dk p) i -> p dk i", p=128)
    w3_r = w3t.rearrange("(dk p) i -> p dk i", p=128)
    w2_r = w2t.rearrange("(mi p) d -> p mi d", p=128)
    ws1_r = ws1t.rearrange("(dk p) i -> p dk i", p=128)
    ws3_r = ws3t.rearrange("(dk p) i -> p dk i", p=128)
    ws2_r = ws2t.rearrange("(mi p) d -> p mi d", p=128)
    ye_r = ye.rearrange("(md p) c -> p md c", p=128)
    ys_r = ys.rearrange("(md p) c -> p md c", p=128)

    with tile.TileContext(nc) as tc:
        with tc.tile_pool(name="bias", bufs=1) as bpool:
            b1_sb = bpool.tile([128, NI], F32, tag="b1")
            nc.sync.dma_start(b1_sb[:], b1.rearrange("(mi p) -> p mi", p=128))
            b3_sb = bpool.tile([128, NI], F32, tag="b3")
            nc.sync.dma_start(b3_sb[:], b3.rearrange("(mi p) -> p mi", p=128))
            b2_sb = bpool.tile([128, ND], F32, tag="b2")
            nc.sync.dma_start(b2_sb[:], b2.rearrange("(md p) -> p md", p=128))
            bs1_sb = bpool.tile([128, NS], F32, tag="bs1")
            nc.sync.dma_start(bs1_sb[:], bs1.rearrange("(mi p) -> p mi", p=128))
            bs3_sb = bpool.tile([128, NS], F32, tag="bs3")
            nc.sync.dma_start(bs3_sb[:], bs3.rearrange("(mi p) -> p mi", p=128))
            bs2_sb = bpool.tile([128, ND], F32, tag="bs2")
            nc.sync.dma_start(bs2_sb[:], bs2.rearrange("(md p) -> p md", p=128))

            # ------------ Phase 1: routed expert (weights resident) --------
            with tc.tile_pool(name="wexp", bufs=1) as wpool, \
                 tc.tile_pool(name="acts", bufs=2) as apool, \
                 tc.tile_pool(name="yout", bufs=3) as ypool, \
                 tc.tile_pool(name="ps", bufs=2, space="PSUM") as pspool:
                w1_sb = wpool.tile([128, ND, INTER], F32R, tag="w1")
                nc.sync.dma_start(w1_sb[:], w1_r)
                w3_sb = wpool.tile([128, ND, INTER], F32R, tag="w3")
                nc.sync.dma_start(w3_sb[:], w3_r)
                w2_sb = wpool.tile([128, NI, DIM], F32R, tag="w2")
                nc.sync.dma_start(w2_sb[:], w2_r)
                for (off, n) in _blocks(C):
                    xb = apool.tile([128, ND, n], F32R, tag="xb",
                                    padded_shape=[128, ND, BLK])
                    nc.sync.dma_start(xb[:], xe_r[:, :, off:off + n])
                    hb = apool.tile([128, NI, n], F32R, tag="hb",
                                    padded_shape=[128, NI, BLK])
                    for mi in range(NI):
                        ps1 = pspool.tile([128, n], F32, tag="ps1",
                                          padded_shape=[128, BLK])
                        ps3 = pspool.tile([128, n], F32, tag="ps3",
                                          padded_shape=[128, BLK])
                        for dk in range(ND):
                            nc.tensor.matmul(
                                ps1[:], w1_sb[:, dk, mi * 128:(mi + 1) * 128],
                                xb[:, dk, :],
                                start=(dk == 0), stop=(dk == ND - 1))
                        for dk in range(ND):
                            nc.tensor.matmul(
                                ps3[:], w3_sb[:, dk, mi * 128:(mi + 1) * 128],
                                xb[:, dk, :],
                                start=(dk == 0), stop=(dk == ND - 1))
                        t1 = apool.tile([128, n], F32, tag="t1",
                                        padded_shape=[128, BLK])
                        nc.scalar.activation(t1[:], ps1[:], SILU,
                                             bias=b1_sb[:, mi:mi + 1])
                        t3 = apool.tile([128, n], F32, tag="t3",
                                        padded_shape=[128, BLK])
                        nc.scalar.activation(t3[:], ps3[:], IDENT,
                                             bias=b3_sb[:, mi:mi + 1])
                        nc.vector.tensor_mul(hb[:, mi, :], t1[:], t3[:])
                    for md in range(ND):
                        psy = pspool.tile([128, n], F32, tag="psy",
                                          padded_shape=[128, BLK])
                        for mi in range(NI):
                            nc.tensor.matmul(
                                psy[:], w2_sb[:, mi, md * 128:(md + 1) * 128],
                                hb[:, mi, :],
                                start=(mi == 0), stop=(mi == NI - 1))
                        yt = ypool.tile([128, n], F32, tag="yt",
                                        padded_shape=[128, BLK])
                        nc.scalar.activation(yt[:], psy[:], IDENT,
                                             bias=b2_sb[:, md:md + 1])
                        nc.sync.dma_start(ye_r[:, md, off:off + n], yt[:])

            # ------------- Phase 2: shared expert (weights streamed) -------
            with tc.tile_pool(name="sacts", bufs=1) as spool, \
                 tc.tile_pool(name="swt", bufs=2) as swpool, \
                 tc.tile_pool(name="stmp", bufs=2) as stpool, \
                 tc.tile_pool(name="syout", bufs=3) as sypool, \
                 tc.tile_pool(name="sps", bufs=2, space="PSUM") as spspool:
                xs_sb = spool.tile([128, ND, TOKS_SHARED], F32R, tag="xs")
                nc.sync.dma_start(xs_sb[:], xs_r)
                hs_sb = spool.tile([128, NS, TOKS_SHARED], F32R, tag="hs")

                for mi in range(NS):
                    wc1 = swpool.tile([128, ND, 128], F32R, tag="wc1")
                    nc.sync.dma_start(wc1[:], ws1_r[:, :, mi * 128:(mi + 1) * 128])
                    wc3 = swpool.tile([128, ND, 128], F32R, tag="wc3")
                    nc.sync.dma_start(wc3[:], ws3_r[:, :, mi * 128:(mi + 1) * 128])
                    for (off, n) in _blocks(TOKS_SHARED):
                        ps1 = spspool.tile([128, n], F32, tag="ps1",
                                           padded_shape=[128, BLK])
                        ps3 = spspool.tile([128, n], F32, tag="ps3",
                                           padded_shape=[128, BLK])
                        for dk in range(ND):
                            nc.tensor.matmul(
                                ps1[:], wc1[:, dk, :],
                                xs_sb[:, dk, off:off + n],
                                start=(dk == 0), stop=(dk == ND - 1))
                        for dk in range(ND):
                            nc.tensor.matmul(
                                ps3[:], wc3[:, dk, :],
                                xs_sb[:, dk, off:off + n],
                                start=(dk == 0), stop=(dk == ND - 1))
                        t1 = stpool.tile([128, n], F32, tag="t1",
                                         padded_shape=[128, BLK])
                        nc.scalar.activation(t1[:], ps1[:], SILU,
                                             bias=bs1_sb[:, mi:mi + 1])
                        t3 = stpool.tile([128, n], F32, tag="t3",
                                         padded_shape=[128, BLK])
                        nc.scalar.activation(t3[:], ps3[:], IDENT,
                                             bias=bs3_sb[:, mi:mi + 1])
                        nc.vector.tensor_mul(hs_sb[:, mi, off:off + n],
                                             t1[:], t3[:])

                for md in range(ND):
                    wc2 = swpool.tile([128, NS, 128], F32R, tag="wc2")
                    nc.sync.dma_start(wc2[:], ws2_r[:, :, md * 128:(md + 1) * 128])
                    for (off, n) in _blocks(TOKS_SHARED):
                        psy = spspool.tile([128, n], F32, tag="psy",
                                           padded_shape=[128, BLK])
                        for mi in range(NS):
                            nc.tensor.matmul(
                                psy[:], wc2[:, mi, :],
                                hs_sb[:, mi, off:off + n],
                                start=(mi == 0), stop=(mi == NS - 1))
                        yt = sypool.tile([128, n], F32, tag="yt",
                                         padded_shape=[128, BLK])
                        nc.scalar.activation(yt[:], psy[:], IDENT,
                                             bias=bs2_sb[:, md:md + 1])
                        nc.sync.dma_start(ys_r[:, md, off:off + n], yt[:])

    nc.compile()
    return nc


def _gate_host(xt, gate_w, gate_b):
    """Softmax gate + top-2 routing, computed in float64 on the host."""
    logits = xt.astype(np.float64) @ gate_w.astype(np.float64).T \
        + gate_b.astype(np.float64)
    m = logits.max(axis=-1, keepdims=True)
    p = np.exp(logits - m)
    scores = p / p.sum(axis=-1, keepdims=True)
    order = np.argsort(-scores, axis=1, kind="stable")
    top_i = order[:, :TOPK]
    top_w = (np.take_along_axis(scores, top_i, axis=1)
             * ROUTE_SCALE).astype(np.float32)
    return top_i, top_w


def run(inputs, trace=False):
    x = np.ascontiguousarray(np.asarray(inputs["x"], dtype=np.float32))
    gate_w = np.asarray(inputs["gate_w"], dtype=np.float32)
    gate_b = np.asarray(inputs["gate_b"], dtype=np.float32)
    w1 = np.asarray(inputs["w1"], dtype=np.float32)
    b1 = np.asarray(inputs["b1"], dtype=np.float32)
    w3 = np.asarray(inputs["w3"], dtype=np.float32)
    b3 = np.asarray(inputs["b3"], dtype=np.float32)
    w2 = np.asarray(inputs["w2"], dtype=np.float32)
    b2 = np.asarray(inputs["b2"], dtype=np.float32)
    ws1 = np.asarray(inputs["ws1"], dtype=np.float32)
    bs1 = np.asarray(inputs["bs1"], dtype=np.float32)
    ws3 = np.asarray(inputs["ws3"], dtype=np.float32)
    bs3 = np.asarray(inputs["bs3"], dtype=np.float32)
    ws2 = np.asarray(inputs["ws2"], dtype=np.float32)
    bs2 = np.asarray(inputs["bs2"], dtype=np.float32)

    xt = x.reshape(T, DIM)
    top_i, top_w = _gate_host(xt, gate_w, gate_b)

    # Dispatch: token lists + gate weights per expert.
    idx, wgt = [], []
    for e in range(E):
        toks = np.nonzero((top_i == e).any(axis=1))[0]
        idx.append(toks)
        slot = (top_i[toks] == e)            # [n_e, TOPK], exactly one True/row
        wgt.append(top_w[toks][slot])

    cmax = max(len(i) for i in idx)
    C = max(256, -(-cmax // 256) * 256)

    ws1t = np.ascontiguousarray(ws1.T)
    ws3t = np.ascontiguousarray(ws3.T)
    ws2t = np.ascontiguousarray(ws2.T)

    in_maps = []
    for e in range(E):
        xe = np.zeros((DIM, C), np.float32)
        xe[:, :len(idx[e])] = xt[idx[e]].T
        sl = slice(TOKS_SHARED * e, TOKS_SHARED * (e + 1))
        in_maps.append({
            "xe": xe,
            "xs": np.ascontiguousarray(xt[sl].T),
            "w1t": np.ascontiguousarray(w1[e].T),
            "w3t": np.ascontiguousarray(w3[e].T),
            "w2t": np.ascontiguousarray(w2[e].T),
            "ws1t": ws1t, "ws3t": ws3t, "ws2t": ws2t,
            "b1": b1[e], "b3": b3[e], "b2": b2[e],
            "bs1": bs1, "bs3": bs3, "bs2": bs2,
        })

    if C not in _program_cache:
        _program_cache[C] = build_program(C)
    nc = _program_cache[C]

    res = bass_utils.run_bass_kernel_spmd(
        nc, in_maps, core_ids=list(range(N_CORES)), trace=trace)

    y = np.empty((T, DIM), np.float32)
    for e in range(E):
        sl = slice(TOKS_SHARED * e, TOKS_SHARED * (e + 1))
        y[sl] = res.results[e]["ys"].T
    for e in range(E):
        ye = res.results[e]["ye"]
        y[idx[e]] += ye[:, :len(idx[e])].T * wgt[e][:, None]
    return y.reshape(B, S, DIM), res


def kernel(**inputs) -> np.ndarray:
    out, _ = run(inputs, trace=False)
    return out


# revision 8
# speedup vs baseline: 1.0294x; 1.0294x over previous
"""MoE (DeepSeek-style) routed+shared expert forward on 8 TRN2 NeuronCores.

Strategy (expert-parallel, host-side dispatch):
  - Host computes the gate (softmax + top-2) in float64 and gathers each
    expert's routed tokens (this is the "all-to-all by routing index" --
    with full inputs on the host, the host does the dispatch).
  - Core e processes expert e's routed tokens (padded to a uniform
    capacity C) through the SwiGLU FFN, plus a 1/8 slice of all tokens
    through the replicated shared-expert MLP.
  - All activations/weights are fed transposed (features on SBUF
    partitions, tokens on the free dim) so the w1/w3 -> swiglu -> w2
    chain needs no on-chip transposes.
  - Matmuls use float32r (full-rate fp32 on the PE array).
  - Host scatters expert outputs back by routing index, scales by the
    gate weights, and adds the shared-expert output.
"""

import sys

if "/opt/trn_rl_repo" not in sys.path:
    sys.path.insert(0, "/opt/trn_rl_repo")

import numpy as np

import concourse.bass as bass
import concourse.tile as tile
from concourse import bacc, mybir
from concourse import bass_utils

B, S, DIM = 4, 2048, 1024
T = B * S
INTER = 1024
E = 8
TOPK = 2
ROUTE_SCALE = 1.0
SHARED_INTER = 2048
N_CORES = 8
TOKS_SHARED = T // N_CORES  # shared-expert tokens per core
BLK = 512

F32 = mybir.dt.float32
F32R = mybir.dt.float32r
SILU = mybir.ActivationFunctionType.Silu
IDENT = mybir.ActivationFunctionType.Identity

_program_cache = {}


def _blocks(total):
    out = []
    o = 0
    while o < total:
        n = min(BLK, total - o)
        out.append((o, n))
        o += n
    return out


def build_program(C):
    """Build the per-core SPMD Bass program for routed capacity C."""
    nc = bacc.Bacc("TRN2", target_bir_lowering=False, debug=False,
                   num_devices=N_CORES)

    def din(name, shape, dt=F32):
        return nc.dram_tensor(name, shape, dt, kind="ExternalInput").ap()

    def dout(name, shape):
        return nc.dram_tensor(name, shape, F32, kind="ExternalOutput").ap()

    xe = din("xe", (DIM, C), F32R)            # routed tokens for this core's expert, transposed
    xs = din("xs", (DIM, TOKS_SHARED), F32R)  # this core's shared-token slice, transposed
    w1t = din("w1t", (DIM, INTER), F32R)      # w1[e].T
    w3t = din("w3t", (DIM, INTER), F32R)
    w2t = din("w2t", (INTER, DIM), F32R)      # w2[e].T
    ws1t = din("ws1t", (DIM, SHARED_INTER), F32R)
    ws3t = din("ws3t", (DIM, SHARED_INTER), F32R)
    ws2t = din("ws2t", (SHARED_INTER, DIM), F32R)
    b1 = din("b1", (INTER,))
    b3 = din("b3", (INTER,))
    b2 = din("b2", (DIM,))
    bs1 = din("bs1", (SHARED_INTER,))
    bs3 = din("bs3", (SHARED_INTER,))
    bs2 = din("bs2", (DIM,))
    ye = dout("ye", (DIM, C))
    ys = dout("ys", (DIM, TOKS_SHARED))

    ND = DIM // 128        # 8 k-tiles over DIM
    NI = INTER // 128      # 8 tiles over INTER
    NS = SHARED_INTER // 128  # 16 tiles over SHARED_INTER

    xe_r = xe.rearrange("(dk p) c -> p dk c", p=128)
    xs_r = xs.rearrange("(dk p) c -> p dk c", p=128)
    w1_r = w1t.rearrange("(dk p) i -> p dk i", p=128)
    w3_r = w3t.rearrange("(dk p) i -> p dk i", p=128)
    w2_r = w2t.rearrange("(mi p) d -> p mi d", p=128)
    ws1_r = ws1t.rearrange("(dk p) i -> p dk i", p=128)
    ws3_r = ws3t.rearrange("(dk p) i -> p dk i", p=128)
    ws2_r = ws2t.rearrange("(mi p) d -> p mi d", p=128)
    ye_r = ye.rearrange("(md p) c -> p md c", p=128)
    ys_r = ys.rearrange("(md p) c -> p md c", p=128)

    with tile.TileContext(nc) as tc:
        with tc.tile_pool(name="bias", bufs=1) as bpool:
            b1_sb = bpool.tile([128, NI], F32, tag="b1")
            nc.sync.dma_start(b1_sb[:], b1.rearrange("(mi p) -> p mi", p=128))
            b3_sb = bpool.tile([128, NI], F32, tag="b3")
            nc.sync.dma_start(b3_sb[:], b3.rearrange("(mi p) -> p mi", p=128))
            b2_sb = bpool.tile([128, ND], F32, tag="b2")
            nc.sync.dma_start(b2_sb[:], b2.rearrange("(md p) -> p md", p=128))
            bs1_sb = bpool.tile([128, NS], F32, tag="bs1")
            nc.sync.dma_start(bs1_sb[:], bs1.rearrange("(mi p) -> p mi", p=128))
            bs3_sb = bpool.tile([128, NS], F32, tag="bs3")
            nc.sync.dma_start(bs3_sb[:], bs3.rearrange("(mi p) -> p mi", p=128))
            bs2_sb = bpool.tile([128, ND], F32, tag="bs2")
            nc.sync.dma_start(bs2_sb[:], bs2.rearrange("(md p) -> p md", p=128))

            # ------------ Phase 1: routed expert (weights resident) --------
            with tc.tile_pool(name="wexp", bufs=1) as wpool, \
                 tc.tile_pool(name="acts", bufs=2) as apool, \
                 tc.tile_pool(name="yout", bufs=3) as ypool, \
                 tc.tile_pool(name="ps", bufs=2, space="PSUM") as pspool:
                # Per-dk split loads so the first matmul only waits for the
                # first 512KB chunk instead of a whole 4MB tensor.
                w1_sb = wpool.tile([128, ND, INTER], F32R, tag="w1")
                w3_sb = wpool.tile([128, ND, INTER], F32R, tag="w3")
                w2_sb = wpool.tile([128, NI, DIM], F32R, tag="w2")
                for dk in range(ND):
                    nc.sync.dma_start(w1_sb[:, dk, :], w1_r[:, dk, :])
                for dk in range(ND):
                    nc.sync.dma_start(w3_sb[:, dk, :], w3_r[:, dk, :])
                for mi in range(NI):
                    nc.sync.dma_start(w2_sb[:, mi, :], w2_r[:, mi, :])
                for (off, n) in _blocks(C):
                    xb = apool.tile([128, ND, n], F32R, tag="xb",
                                    padded_shape=[128, ND, BLK])
                    for dk in range(ND):
                        nc.sync.dma_start(xb[:, dk, :], xe_r[:, dk, off:off + n])
                    hb = apool.tile([128, NI, n], F32R, tag="hb",
                                    padded_shape=[128, NI, BLK])
                    for mi in range(NI):
                        ps1 = pspool.tile([128, n], F32, tag="ps1",
                                          padded_shape=[128, BLK])
                        ps3 = pspool.tile([128, n], F32, tag="ps3",
                                          padded_shape=[128, BLK])
                        for dk in range(ND):
                            nc.tensor.matmul(
                                ps1[:], w1_sb[:, dk, mi * 128:(mi + 1) * 128],
                                xb[:, dk, :],
                                start=(dk == 0), stop=(dk == ND - 1))
                        for dk in range(ND):
                            nc.tensor.matmul(
                                ps3[:], w3_sb[:, dk, mi * 128:(mi + 1) * 128],
                                xb[:, dk, :],
                                start=(dk == 0), stop=(dk == ND - 1))
                        t1 = apool.tile([128, n], F32, tag="t1",
                                        padded_shape=[128, BLK])
                        nc.scalar.activation(t1[:], ps1[:], SILU,
                                             bias=b1_sb[:, mi:mi + 1])
                        t3 = apool.tile([128, n], F32, tag="t3",
                                        padded_shape=[128, BLK])
                        nc.scalar.activation(t3[:], ps3[:], IDENT,
                                             bias=b3_sb[:, mi:mi + 1])
                        nc.vector.tensor_mul(hb[:, mi, :], t1[:], t3[:])
                    for md in range(ND):
                        psy = pspool.tile([128, n], F32, tag="psy",
                                          padded_shape=[128, BLK])
                        for mi in range(NI):
                            nc.tensor.matmul(
                                psy[:], w2_sb[:, mi, md * 128:(md + 1) * 128],
                                hb[:, mi, :],
                                start=(mi == 0), stop=(mi == NI - 1))
                        yt = ypool.tile([128, n], F32, tag="yt",
                                        padded_shape=[128, BLK])
                        nc.scalar.activation(yt[:], psy[:], IDENT,
                                             bias=b2_sb[:, md:md + 1])
                        nc.sync.dma_start(ye_r[:, md, off:off + n], yt[:])

            # ------------- Phase 2: shared expert (weights streamed) -------
            with tc.tile_pool(name="sacts", bufs=1) as spool, \
                 tc.tile_pool(name="swt", bufs=2) as swpool, \
                 tc.tile_pool(name="stmp", bufs=2) as stpool, \
                 tc.tile_pool(name="syout", bufs=3) as sypool, \
                 tc.tile_pool(name="sps", bufs=2, space="PSUM") as spspool:
                xs_sb = spool.tile([128, ND, TOKS_SHARED], F32R, tag="xs")
                for dk in range(ND):
                    nc.sync.dma_start(xs_sb[:, dk, :], xs_r[:, dk, :])
                hs_sb = spool.tile([128, NS, TOKS_SHARED], F32R, tag="hs")

                for mi in range(NS):
                    wc1 = swpool.tile([128, ND, 128], F32R, tag="wc1")
                    nc.sync.dma_start(wc1[:], ws1_r[:, :, mi * 128:(mi + 1) * 128])
                    wc3 = swpool.tile([128, ND, 128], F32R, tag="wc3")
                    nc.sync.dma_start(wc3[:], ws3_r[:, :, mi * 128:(mi + 1) * 128])
                    for (off, n) in _blocks(TOKS_SHARED):
                        ps1 = spspool.tile([128, n], F32, tag="ps1",
                                           padded_shape=[128, BLK])
                        ps3 = spspool.tile([128, n], F32, tag="ps3",
                                           padded_shape=[128, BLK])
                        for dk in range(ND):
                            nc.tensor.matmul(
                                ps1[:], wc1[:, dk, :],
                                xs_sb[:, dk, off:off + n],
                                start=(dk == 0), stop=(dk == ND - 1))
                        for dk in range(ND):
                            nc.tensor.matmul(
                                ps3[:], wc3[:, dk, :],
                                xs_sb[:, dk, off:off + n],
                                start=(dk == 0), stop=(dk == ND - 1))
                        t1 = stpool.tile([128, n], F32, tag="t1",
                                         padded_shape=[128, BLK])
                        nc.scalar.activation(t1[:], ps1[:], SILU,
                                             bias=bs1_sb[:, mi:mi + 1])
                        t3 = stpool.tile([128, n], F32, tag="t3",
                                         padded_shape=[128, BLK])
                        nc.scalar.activation(t3[:], ps3[:], IDENT,
                                             bias=bs3_sb[:, mi:mi + 1])
                        nc.vector.tensor_mul(hs_sb[:, mi, off:off + n],
                                             t1[:], t3[:])

                for md in range(ND):
                    wc2 = swpool.tile([128, NS, 128], F32R, tag="wc2")
                    nc.sync.dma_start(wc2[:], ws2_r[:, :, md * 128:(md + 1) * 128])
                    for (off, n) in _blocks(TOKS_SHARED):
                        psy = spspool.tile([128, n], F32, tag="psy",
                                           padded_shape=[128, BLK])
                        for mi in range(NS):
                            nc.tensor.matmul(
                                psy[:], wc2[:, mi, :],
                                hs_sb[:, mi, off:off + n],
                                start=(mi == 0), stop=(mi == NS - 1))
                        yt = sypool.tile([128, n], F32, tag="yt",
                                         padded_shape=[128, BLK])
                        nc.scalar.activation(yt[:], psy[:], IDENT,
                                             bias=bs2_sb[:, md:md + 1])
                        nc.sync.dma_start(ys_r[:, md, off:off + n], yt[:])

    nc.compile()
    return nc


def _gate_host(xt, gate_w, gate_b):
    """Softmax gate + top-2 routing, computed in float64 on the host."""
    logits = xt.astype(np.float64) @ gate_w.astype(np.float64).T \
        + gate_b.astype(np.float64)
    m = logits.max(axis=-1, keepdims=True)
    p = np.exp(logits - m)
    scores = p / p.sum(axis=-1, keepdims=True)
    order = np.argsort(-scores, axis=1, kind="stable")
    top_i = order[:, :TOPK]
    top_w = (np.take_along_axis(scores, top_i, axis=1)
             * ROUTE_SCALE).astype(np.float32)
    return top_i, top_w


def run(inputs, trace=False):
    x = np.ascontiguousarray(np.asarray(inputs["x"], dtype=np.float32))
    gate_w = np.asarray(inputs["gate_w"], dtype=np.float32)
    gate_b = np.asarray(inputs["gate_b"], dtype=np.float32)
    w1 = np.asarray(inputs["w1"], dtype=np.float32)
    b1 = np.asarray(inputs["b1"], dtype=np.float32)
    w3 = np.asarray(inputs["w3"], dtype=np.float32)
    b3 = np.asarray(inputs["b3"], dtype=np.float32)
    w2 = np.asarray(inputs["w2"], dtype=np.float32)
    b2 = np.asarray(inputs["b2"], dtype=np.float32)
    ws1 = np.asarray(inputs["ws1"], dtype=np.float32)
    bs1 = np.asarray(inputs["bs1"], dtype=np.float32)
    ws3 = np.asarray(inputs["ws3"], dtype=np.float32)
    bs3 = np.asarray(inputs["bs3"], dtype=np.float32)
    ws2 = np.asarray(inputs["ws2"], dtype=np.float32)
    bs2 = np.asarray(inputs["bs2"], dtype=np.float32)

    xt = x.reshape(T, DIM)
    top_i, top_w = _gate_host(xt, gate_w, gate_b)

    # Dispatch: token lists + gate weights per expert.
    idx, wgt = [], []
    for e in range(E):
        toks = np.nonzero((top_i == e).any(axis=1))[0]
        idx.append(toks)
        slot = (top_i[toks] == e)            # [n_e, TOPK], exactly one True/row
        wgt.append(top_w[toks][slot])

    cmax = max(len(i) for i in idx)
    C = max(256, -(-cmax // 256) * 256)

    ws1t = np.ascontiguousarray(ws1.T)
    ws3t = np.ascontiguousarray(ws3.T)
    ws2t = np.ascontiguousarray(ws2.T)

    in_maps = []
    for e in range(E):
        xe = np.zeros((DIM, C), np.float32)
        xe[:, :len(idx[e])] = xt[idx[e]].T
        sl = slice(TOKS_SHARED * e, TOKS_SHARED * (e + 1))
        in_maps.append({
            "xe": xe,
            "xs": np.ascontiguousarray(xt[sl].T),
            "w1t": np.ascontiguousarray(w1[e].T),
            "w3t": np.ascontiguousarray(w3[e].T),
            "w2t": np.ascontiguousarray(w2[e].T),
            "ws1t": ws1t, "ws3t": ws3t, "ws2t": ws2t,
            "b1": b1[e], "b3": b3[e], "b2": b2[e],
            "bs1": bs1, "bs3": bs3, "bs2": bs2,
        })

    if C not in _program_cache:
        _program_cache[C] = build_program(C)
    nc = _program_cache[C]

    res = bass_utils.run_bass_kernel_spmd(
        nc, in_maps, core_ids=list(range(N_CORES)), trace=trace)

    y = np.empty((T, DIM), np.float32)
    for e in range(E):
        sl = slice(TOKS_SHARED * e, TOKS_SHARED * (e + 1))
        y[sl] = res.results[e]["ys"].T
    for e in range(E):
        ye = res.results[e]["ye"]
        y[idx[e]] += ye[:, :len(idx[e])].T * wgt[e][:, None]
    return y.reshape(B, S, DIM), res


def kernel(**inputs) -> np.ndarray:
    out, _ = run(inputs, trace=False)
    return out


# revision 12
# speedup vs baseline: 1.0879x; 1.0568x over previous
"""MoE (DeepSeek-style) routed+shared expert forward on 8 TRN2 NeuronCores.

Strategy (expert-parallel, host-side dispatch):
  - Host computes the gate (softmax + top-2) in float64 and gathers each
    expert's routed tokens (this is the "all-to-all by routing index" --
    with full inputs on the host, the host does the dispatch).
  - Core e processes expert e's routed tokens (padded to a uniform
    capacity C) through the SwiGLU FFN, plus a 1/8 slice of all tokens
    through the replicated shared-expert MLP.
  - All activations/weights are fed transposed (features on SBUF
    partitions, tokens on the free dim) so the w1/w3 -> swiglu -> w2
    chain needs no on-chip transposes.
  - Matmuls use float32r (full-rate fp32 on the PE array).
  - Host scatters expert outputs back by routing index, scales by the
    gate weights, and adds the shared-expert output.
"""

import sys

if "/opt/trn_rl_repo" not in sys.path:
    sys.path.insert(0, "/opt/trn_rl_repo")

import ml_dtypes
import numpy as np

import concourse.bass as bass
import concourse.tile as tile
from concourse import bacc, mybir
from concourse import bass_utils

B, S, DIM = 4, 2048, 1024
T = B * S
INTER = 1024
E = 8
TOPK = 2
ROUTE_SCALE = 1.0
SHARED_INTER = 2048
N_CORES = 8
TOKS_SHARED = T // N_CORES  # shared-expert tokens per core
BLK = 512

F32 = mybir.dt.float32
F32R = mybir.dt.float32r
BF16 = mybir.dt.bfloat16
SILU = mybir.ActivationFunctionType.Silu
IDENT = mybir.ActivationFunctionType.Identity

_program_cache = {}


def _blocks(total):
    out = []
    o = 0
    while o < total:
        n = min(BLK, total - o)
        out.append((o, n))
        o += n
    return out


def build_program(C):
    """Build the per-core SPMD Bass program for routed capacity C.

    Phase 1 (routed expert): fp32r matmuls, w1/w3/w2 resident in SBUF.
    Phase 2 (shared expert): bf16 matmuls, ws1/ws3/ws2 resident in SBUF,
    tokens processed in two halves of 512. Each weight/activation chunk
    is a separate tile so matmuls depend only on the chunk they read;
    DMA issue order matches PE consumption order.
    """
    nc = bacc.Bacc("TRN2", target_bir_lowering=False, debug=False,
                   num_devices=N_CORES)

    def din(name, shape, dt=F32):
        return nc.dram_tensor(name, shape, dt, kind="ExternalInput").ap()

    def dout(name, shape):
        return nc.dram_tensor(name, shape, F32, kind="ExternalOutput").ap()

    xe = din("xe", (DIM, C), F32R)            # routed tokens, transposed
    xs = din("xs", (DIM, TOKS_SHARED), BF16)  # shared-token slice, transposed
    w1t = din("w1t", (DIM, INTER), F32R)      # w1[e].T
    w3t = din("w3t", (DIM, INTER), F32R)
    w2t = din("w2t", (INTER, DIM), F32R)      # w2[e].T
    ws1t = din("ws1t", (DIM, SHARED_INTER), BF16)
    ws3t = din("ws3t", (DIM, SHARED_INTER), BF16)
    ws2t = din("ws2t", (SHARED_INTER, DIM), BF16)
    b1 = din("b1", (INTER,))
    b3 = din("b3", (INTER,))
    b2 = din("b2", (DIM,))
    bs1 = din("bs1", (SHARED_INTER,))
    bs3 = din("bs3", (SHARED_INTER,))
    bs2 = din("bs2", (DIM,))
    ye = dout("ye", (DIM, C))
    ys = dout("ys", (DIM, TOKS_SHARED))

    ND = DIM // 128           # 8 k-tiles over DIM
    NI = INTER // 128         # 8 tiles over INTER
    NS = SHARED_INTER // 128  # 16 tiles over SHARED_INTER

    xe_r = xe.rearrange("(dk p) c -> p dk c", p=128)
    xs_r = xs.rearrange("(dk p) c -> p dk c", p=128)
    w1_r = w1t.rearrange("(dk p) i -> p dk i", p=128)
    w3_r = w3t.rearrange("(dk p) i -> p dk i", p=128)
    w2_r = w2t.rearrange("(mi p) d -> p mi d", p=128)
    ws1_r = ws1t.rearrange("(dk p) i -> p dk i", p=128)
    ws3_r = ws3t.rearrange("(dk p) i -> p dk i", p=128)
    ws2_r = ws2t.rearrange("(mi p) d -> p mi d", p=128)
    ye_r = ye.rearrange("(md p) c -> p md c", p=128)
    ys_r = ys.rearrange("(md p) c -> p md c", p=128)

    with tile.TileContext(nc) as tc:
        with tc.tile_pool(name="bias", bufs=1) as bpool, \
             tc.tile_pool(name="tmp", bufs=2) as tpool, \
             tc.tile_pool(name="yout", bufs=3) as ypool, \
             tc.tile_pool(name="ps", bufs=2, space="PSUM") as pspool:
            b1_sb = bpool.tile([128, NI], F32, tag="b1")
            nc.sync.dma_start(b1_sb[:], b1.rearrange("(mi p) -> p mi", p=128))
            b3_sb = bpool.tile([128, NI], F32, tag="b3")
            nc.sync.dma_start(b3_sb[:], b3.rearrange("(mi p) -> p mi", p=128))
            b2_sb = bpool.tile([128, ND], F32, tag="b2")
            nc.sync.dma_start(b2_sb[:], b2.rearrange("(md p) -> p md", p=128))
            bs1_sb = bpool.tile([128, NS], F32, tag="bs1")
            nc.sync.dma_start(bs1_sb[:], bs1.rearrange("(mi p) -> p mi", p=128))
            bs3_sb = bpool.tile([128, NS], F32, tag="bs3")
            nc.sync.dma_start(bs3_sb[:], bs3.rearrange("(mi p) -> p mi", p=128))
            bs2_sb = bpool.tile([128, ND], F32, tag="bs2")
            nc.sync.dma_start(bs2_sb[:], bs2.rearrange("(md p) -> p md", p=128))

            # ---------- Phase 1: routed expert (fp32r, weights resident) ----
            with tc.tile_pool(name="wexp", bufs=1) as wpool, \
                 tc.tile_pool(name="xbp", bufs=2) as xpool, \
                 tc.tile_pool(name="hbp", bufs=1) as hpool:
                blocks = _blocks(C)
                # First block's activations + w1 interleaved, so the first
                # matmuls are gated on ~1MB, then w3, w2 stream behind.
                xb0 = [xpool.tile([128, blocks[0][1]], F32R, tag=f"xb{dk}",
                                  name=f"xb{dk}", padded_shape=[128, BLK])
                       for dk in range(ND)]
                w1_sb, w3_sb, w2_sb = [], [], []
                for dk in range(ND):
                    nc.sync.dma_start(xb0[dk][:], xe_r[:, dk, 0:blocks[0][1]])
                    t = wpool.tile([128, INTER], F32R, tag=f"w1_{dk}")
                    nc.sync.dma_start(t[:], w1_r[:, dk, :])
                    w1_sb.append(t)
                for dk in range(ND):
                    t = wpool.tile([128, INTER], F32R, tag=f"w3_{dk}")
                    nc.sync.dma_start(t[:], w3_r[:, dk, :])
                    w3_sb.append(t)
                for mi in range(NI):
                    t = wpool.tile([128, DIM], F32R, tag=f"w2_{mi}")
                    nc.sync.dma_start(t[:], w2_r[:, mi, :])
                    w2_sb.append(t)

                for bi, (off, n) in enumerate(blocks):
                    if bi == 0:
                        xb = xb0
                    else:
                        xb = [xpool.tile([128, n], F32R, tag=f"xb{dk}",
                                         name=f"xb{dk}",
                                         padded_shape=[128, BLK])
                              for dk in range(ND)]
                        for dk in range(ND):
                            nc.sync.dma_start(xb[dk][:], xe_r[:, dk, off:off + n])
                    hb = [hpool.tile([128, n], F32R, tag=f"hb{mi}",
                                     name=f"hb{mi}", padded_shape=[128, BLK])
                          for mi in range(NI)]
                    for mi in range(NI):
                        ps1 = pspool.tile([128, n], F32, tag="ps1",
                                          padded_shape=[128, BLK])
                        ps3 = pspool.tile([128, n], F32, tag="ps3",
                                          padded_shape=[128, BLK])
                        for dk in range(ND):
                            nc.tensor.matmul(
                                ps1[:], w1_sb[dk][:, mi * 128:(mi + 1) * 128],
                                xb[dk][:],
                                start=(dk == 0), stop=(dk == ND - 1))
                        for dk in range(ND):
                            nc.tensor.matmul(
                                ps3[:], w3_sb[dk][:, mi * 128:(mi + 1) * 128],
                                xb[dk][:],
                                start=(dk == 0), stop=(dk == ND - 1))
                        t1 = tpool.tile([128, n], F32, tag="t1",
                                        padded_shape=[128, BLK])
                        nc.scalar.activation(t1[:], ps1[:], SILU,
                                             bias=b1_sb[:, mi:mi + 1])
                        t3 = tpool.tile([128, n], F32, tag="t3",
                                        padded_shape=[128, BLK])
                        nc.scalar.activation(t3[:], ps3[:], IDENT,
                                             bias=b3_sb[:, mi:mi + 1])
                        nc.vector.tensor_mul(hb[mi][:], t1[:], t3[:])
                    for md in range(ND):
                        psy = pspool.tile([128, n], F32, tag="psy",
                                          padded_shape=[128, BLK])
                        for mi in range(NI):
                            nc.tensor.matmul(
                                psy[:], w2_sb[mi][:, md * 128:(md + 1) * 128],
                                hb[mi][:],
                                start=(mi == 0), stop=(mi == NI - 1))
                        yt = ypool.tile([128, n], F32, tag="yt",
                                        padded_shape=[128, BLK])
                        nc.scalar.activation(yt[:], psy[:], IDENT,
                                             bias=b2_sb[:, md:md + 1])
                        nc.sync.dma_start(ye_r[:, md, off:off + n], yt[:])

            # ---------- Phase 2: shared expert (bf16, weights resident) -----
            with tc.tile_pool(name="wsh", bufs=1) as wspool, \
                 tc.tile_pool(name="hsp", bufs=1) as hspool, \
                 tc.tile_pool(name="stmp", bufs=2) as stpool:
                xs_sb, ws1_sb, ws3_sb, ws2_sb = [], [], [], []
                for dk in range(ND):
                    t = wspool.tile([128, TOKS_SHARED], BF16, tag=f"xs{dk}")
                    nc.sync.dma_start(t[:], xs_r[:, dk, :])
                    xs_sb.append(t)
                    t = wspool.tile([128, SHARED_INTER], BF16, tag=f"ws1_{dk}")
                    nc.sync.dma_start(t[:], ws1_r[:, dk, :])
                    ws1_sb.append(t)
                for dk in range(ND):
                    t = wspool.tile([128, SHARED_INTER], BF16, tag=f"ws3_{dk}")
                    nc.sync.dma_start(t[:], ws3_r[:, dk, :])
                    ws3_sb.append(t)
                for mi in range(NS):
                    t = wspool.tile([128, DIM], BF16, tag=f"ws2_{mi}")
                    nc.sync.dma_start(t[:], ws2_r[:, mi, :])
                    ws2_sb.append(t)

                for (off, n) in _blocks(TOKS_SHARED):
                    hs = [hspool.tile([128, n], BF16, tag=f"hs{mi}",
                                      name=f"hs{mi}", padded_shape=[128, BLK])
                          for mi in range(NS)]
                    for mi in range(NS):
                        ps1 = pspool.tile([128, n], F32, tag="ps1",
                                          padded_shape=[128, BLK])
                        ps3 = pspool.tile([128, n], F32, tag="ps3",
                                          padded_shape=[128, BLK])
                        for dk in range(ND):
                            nc.tensor.matmul(
                                ps1[:], ws1_sb[dk][:, mi * 128:(mi + 1) * 128],
                                xs_sb[dk][:, off:off + n],
                                start=(dk == 0), stop=(dk == ND - 1))
                        for dk in range(ND):
                            nc.tensor.matmul(
                                ps3[:], ws3_sb[dk][:, mi * 128:(mi + 1) * 128],
                                xs_sb[dk][:, off:off + n],
                                start=(dk == 0), stop=(dk == ND - 1))
                        t1 = stpool.tile([128, n], BF16, tag="t1s",
                                         padded_shape=[128, BLK])
                        nc.scalar.activation(t1[:], ps1[:], SILU,
                                             bias=bs1_sb[:, mi:mi + 1])
                        t3 = stpool.tile([128, n], BF16, tag="t3s",
                                         padded_shape=[128, BLK])
                        nc.scalar.activation(t3[:], ps3[:], IDENT,
                                             bias=bs3_sb[:, mi:mi + 1])
                        nc.vector.tensor_mul(hs[mi][:], t1[:], t3[:])
                    for md in range(ND):
                        psy = pspool.tile([128, n], F32, tag="psy",
                                          padded_shape=[128, BLK])
                        for mi in range(NS):
                            nc.tensor.matmul(
                                psy[:], ws2_sb[mi][:, md * 128:(md + 1) * 128],
                                hs[mi][:],
                                start=(mi == 0), stop=(mi == NS - 1))
                        yt = ypool.tile([128, n], F32, tag="yt",
                                        padded_shape=[128, BLK])
                        nc.scalar.activation(yt[:], psy[:], IDENT,
                                             bias=bs2_sb[:, md:md + 1])
                        nc.sync.dma_start(ys_r[:, md, off:off + n], yt[:])

    nc.compile()
    return nc


def _gate_host(xt, gate_w, gate_b):
    """Softmax gate + top-2 routing, computed in float64 on the host."""
    logits = xt.astype(np.float64) @ gate_w.astype(np.float64).T \
        + gate_b.astype(np.float64)
    m = logits.max(axis=-1, keepdims=True)
    p = np.exp(logits - m)
    scores = p / p.sum(axis=-1, keepdims=True)
    order = np.argsort(-scores, axis=1, kind="stable")
    top_i = order[:, :TOPK]
    top_w = (np.take_along_axis(scores, top_i, axis=1)
             * ROUTE_SCALE).astype(np.float32)
    return top_i, top_w


def run(inputs, trace=False):
    x = np.ascontiguousarray(np.asarray(inputs["x"], dtype=np.float32))
    gate_w = np.asarray(inputs["gate_w"], dtype=np.float32)
    gate_b = np.asarray(inputs["gate_b"], dtype=np.float32)
    w1 = np.asarray(inputs["w1"], dtype=np.float32)
    b1 = np.asarray(inputs["b1"], dtype=np.float32)
    w3 = np.asarray(inputs["w3"], dtype=np.float32)
    b3 = np.asarray(inputs["b3"], dtype=np.float32)
    w2 = np.asarray(inputs["w2"], dtype=np.float32)
    b2 = np.asarray(inputs["b2"], dtype=np.float32)
    ws1 = np.asarray(inputs["ws1"], dtype=np.float32)
    bs1 = np.asarray(inputs["bs1"], dtype=np.float32)
    ws3 = np.asarray(inputs["ws3"], dtype=np.float32)
    bs3 = np.asarray(inputs["bs3"], dtype=np.float32)
    ws2 = np.asarray(inputs["ws2"], dtype=np.float32)
    bs2 = np.asarray(inputs["bs2"], dtype=np.float32)

    xt = x.reshape(T, DIM)
    top_i, top_w = _gate_host(xt, gate_w, gate_b)

    # Dispatch: token lists + gate weights per expert.
    idx, wgt = [], []
    for e in range(E):
        toks = np.nonzero((top_i == e).any(axis=1))[0]
        idx.append(toks)
        slot = (top_i[toks] == e)            # [n_e, TOPK], exactly one True/row
        wgt.append(top_w[toks][slot])

    cmax = max(len(i) for i in idx)
    C = max(256, -(-cmax // 256) * 256)

    bf16 = ml_dtypes.bfloat16
    ws1t = np.ascontiguousarray(ws1.T).astype(bf16)
    ws3t = np.ascontiguousarray(ws3.T).astype(bf16)
    ws2t = np.ascontiguousarray(ws2.T).astype(bf16)
    xt_bf = xt.astype(bf16)

    in_maps = []
    for e in range(E):
        xe = np.zeros((DIM, C), np.float32)
        xe[:, :len(idx[e])] = xt[idx[e]].T
        sl = slice(TOKS_SHARED * e, TOKS_SHARED * (e + 1))
        in_maps.append({
            "xe": xe,
            "xs": np.ascontiguousarray(xt_bf[sl].T),
            "w1t": np.ascontiguousarray(w1[e].T),
            "w3t": np.ascontiguousarray(w3[e].T),
            "w2t": np.ascontiguousarray(w2[e].T),
            "ws1t": ws1t, "ws3t": ws3t, "ws2t": ws2t,
            "b1": b1[e], "b3": b3[e], "b2": b2[e],
            "bs1": bs1, "bs3": bs3, "bs2": bs2,
        })

    if C not in _program_cache:
        _program_cache[C] = build_program(C)
    nc = _program_cache[C]

    res = bass_utils.run_bass_kernel_spmd(
        nc, in_maps, core_ids=list(range(N_CORES)), trace=trace)

    y = np.empty((T, DIM), np.float32)
    for e in range(E):
        sl = slice(TOKS_SHARED * e, TOKS_SHARED * (e + 1))
        y[sl] = res.results[e]["ys"].T
    for e in range(E):
        ye = res.results[e]["ye"]
        y[idx[e]] += ye[:, :len(idx[e])].T * wgt[e][:, None]
    return y.reshape(B, S, DIM), res


def kernel(**inputs) -> np.ndarray:
    out, _ = run(inputs, trace=False)
    return out
